# revision 72
# baseline (speedup 1.0000x reference)
"""Low-rank layer y = (U^T V) @ x computed as y = U^T @ (V @ x).

Full problem: x [8192, 4096] f32, U/V [8, 8192] f32, y [8192, 4096] f32.
Sharding: batch (columns of x) split across 8 NeuronCores, 512 per core.

All device I/O and matmul operands are fp16 (the 2e-2 rel-err budget
dwarfs fp16's ~1e-3): the host casts x/U/V to fp16 before upload and
upcasts y after download, so HBM traffic per core is 8 MiB in + 8 MiB
out instead of 16+16, and the PE streams at 1 cycle/row instead of
fp32's 4. PSUM accumulation stays fp32; the PSUM->SBUF copies cast.

The 512 batch columns are processed as two pipelined halves of 256 so
the serial DMA engine never idles at a phase boundary:

  loads:  x H0 g0 | vt u | x H0 rest | x H1   (tail groups tiny: gate T0)
  PE:     ph1(H0) | ph2(H0) pairs interleaved 1:1 with ph1(H1) | ph2(H1)
  stores: H0 (queued behind H1 loads)         | H1

T(half) = V @ x_half accumulates in its own PSUM bank; y matmul pairs
[8,128]^T @ [8,256] land two-to-a-bank and drain via one [128,512]
cast-copy alternating ACT/DVE (the phase-2 throughput limit, ~640 ns
per pair across both engines).

Engines execute ready instructions past <=4 parked blocked ones, but
the 5th blocked instruction stalls the whole sequencer -- so the
half-1 loads lead with small groups, keeping the interleaved T1
matmuls fed. No filler matmuls anywhere: the PE tracks the DMA feed
with ~40% duty and every added instruction just delays the real ones.
"""

import numpy as np

L = 8192
RANK = 8
BATCH = 4096
NCORES = 8
BS = BATCH // NCORES  # 512 batch columns per core
HB = BS // 2          # 256 columns per pipelined half
P = 128               # SBUF partitions
NCHUNK = L // P       # 64 row-chunks of 128
NPAIR = NCHUNK // 2

# Input DMA group sizes (chunks) for half 0: big groups amortize the
# serial ~625 ns/DMA HWDGE descriptor generation; the tiny tail gates T0
# minimally (T0 waits on the last chunk's matmul).
XGROUPS = [8, 16, 16, 12, 8, 2, 1, 1]
# Half 1 leads with small groups: its chunks are consumed 1-per-pair by
# the interleave from t~15.4us at ~3 chunks/us, and a big leading group
# would land too late, parking the interleaved matmuls (>=4 parked stalls
# the whole PE sequencer).
XGROUPS1 = [4, 8, 16, 16, 16, 4]
assert sum(XGROUPS) == NCHUNK and sum(XGROUPS1) == NCHUNK
# Half-1 T matmuls interleaved into half-0 phase 2: one per pair keeps the
# pair cadence at ~321 ns — right at the copy engines' ~330 ns/pair limit —
# so half-0 store readiness never trails the DMA. The other 32 run as a
# solid block right after (PE is otherwise idle there; loads have landed).
INTER1_PER_PAIR = 1
# Output store sizes (chunks): both halves' copies run far ahead of the
# DMA engine (it's busy with loads until ~26us), so stores are plain
# 16-chunk (1 MiB) transfers. HWDGE (sync) stores: the per-store issue
# cost is ~625 ns vs the Pool/SWDGE path's ~1.1-1.6 us descriptor
# generation, and SP's HWDGE FIFO enforces the load-before-store order
# this schedule wants anyway.
YGROUPS = [16, 16, 16, 16]
STORE_HWDGE = True
YGROUPS1 = [16, 16, 16, 16]
assert sum(YGROUPS) == NCHUNK and sum(YGROUPS1) == NCHUNK

_NC = None  # cached compiled Bass module


def _body(tc, nc, x, vt, u, y, mybir):
    from contextlib import ExitStack

    f16 = mybir.dt.float16
    f32 = mybir.dt.float32
    x3 = x.rearrange("(n p) b -> p n b", p=P)   # [128, 64, 512] view of DRAM
    y3 = y.rearrange("(n p) b -> p n b", p=P)

    with ExitStack() as ctx:
        const = ctx.enter_context(tc.tile_pool(name="const", bufs=1))
        xbuf = ctx.enter_context(tc.tile_pool(name="xbuf", bufs=1))
        ybuf = ctx.enter_context(tc.tile_pool(name="ybuf", bufs=1))
        tpsum = ctx.enter_context(tc.tile_pool(name="tpsum", bufs=1, space="PSUM"))
        ypsum = ctx.enter_context(tc.tile_pool(name="ypsum", bufs=5, space="PSUM"))

        vt_sb = const.tile([P, NCHUNK * RANK], f16)   # vt[p, n*8+r] = V[r, n*128+p]
        u_sb = const.tile([RANK, L], f16)
        t_sb = [const.tile([RANK, HB], f16, tag=f"t{h}", name=f"t_sb{h}")
                for h in range(2)]

        # x and y live in SBUF whole: every DMA and copy touches a distinct
        # slice, so no buffer-reuse hazards anywhere (this walrus build
        # encodes at most ONE sync wait per instruction).
        x_sb = xbuf.tile([P, NCHUNK * BS], f16)
        y_sb = ybuf.tile([P, NCHUNK * BS], f16)

        # One PSUM bank absorbing the warm-up matmuls.
        warm = tpsum.tile([P, BS], f32, tag="warm")

        # Absorbs the vt DMA wait; ACT's first op pays a ~245 ns function
        # table load, absorbed into a scratch copy (overwritten later).
        nc.tensor.matmul(warm[:RANK, 0:RANK], vt_sb[:, 0:RANK],
                         vt_sb[:, 0:RANK], start=True, stop=True)
        nc.scalar.copy(t_sb[0][:, 0:RANK], vt_sb[0:RANK, 0:RANK])

        t_ps = [tpsum.tile([RANK, HB], f32, tag=f"tp{h}", name=f"t_ps{h}")
                for h in range(2)]

        def ph1_mm(h, n):
            nc.tensor.matmul(
                t_ps[h][:],
                vt_sb[:, n * RANK:(n + 1) * RANK],          # lhsT [128, 8]
                x_sb[:, n * BS + h * HB:n * BS + (h + 1) * HB],  # rhs [128, 256]
                start=(n == 0),
                stop=(n == NCHUNK - 1),
            )

        def x_load(h, n0, g):
            nc.sync.dma_start(
                x_sb[:].rearrange("p (n b) -> p n b", b=BS)[
                    :, n0:n0 + g, h * HB:(h + 1) * HB],
                x3[:, n0:n0 + g, h * HB:(h + 1) * HB],
            )

        # ---- Half-0 phase 1: loads + T0 matmuls. Load order g0 (8
        # chunks), vt, u, g1...: a tiny lead DMA cannot hide the next one's
        # ~625 ns serial HWDGE descriptor generation, so the big g0 leads
        # and the two small loads draft behind its stream. The PE has ~2x
        # slack over the feed rate, so starting its matmuls at vt-landing
        # (~3.8us) costs nothing. ----
        n0 = 0
        for gi, g in enumerate(XGROUPS):
            x_load(0, n0, g)
            if gi == 0:
                # Issued from ACT's sequencer (also HWDGE): keeps the SP
                # sequencer's ~650 ns/DMA issue pipe clear of small loads.
                nc.scalar.dma_start(vt_sb[:], vt[:])
                nc.scalar.dma_start(u_sb[:], u[:])
            for c in range(g):
                ph1_mm(0, n0 + c)
            n0 += g
        # ---- Half-1 loads: queue right behind half-0's so the DMA engine
        # streams them while half-0 computes its boundary + stores. ----
        n0 = 0
        for g in XGROUPS1:
            x_load(1, n0, g)
            n0 += g

        # Absorb the u DMA wait off the critical path.
        nc.tensor.matmul(warm[:, 0:RANK], u_sb[:, 0:P], u_sb[:, 0:RANK],
                         start=True, stop=True)

        # T0 to SBUF, casting f32 -> f16, split across both copy engines.
        nc.vector.tensor_copy(t_sb[0][:, 0:HB // 2], t_ps[0][:, 0:HB // 2])
        nc.scalar.copy(t_sb[0][:, HB // 2:], t_ps[0][:, HB // 2:])

        def ph2(h, interleave):
            """y pairs for half h; interleave(pair) emits extra PE work."""
            for pair in range(NPAIR):
                y_ps = ypsum.tile([P, 2 * HB], f32, tag="yp")
                for half in range(2):
                    n = 2 * pair + half
                    nc.tensor.matmul(
                        y_ps[:, half * HB:(half + 1) * HB],
                        u_sb[:, n * P:(n + 1) * P],  # lhsT [8, 128]
                        t_sb[h][:],                  # rhs  [8, 256]
                        start=True,
                        stop=True,
                    )
                interleave(pair)
                # Cast-copy both chunks' columns of this half in one op.
                if pair % 2 == 0:
                    nc.scalar.copy(
                        y_sb[:].rearrange("p (n b) -> p n b", b=BS)[
                            :, 2 * pair:2 * pair + 2, h * HB:(h + 1) * HB],
                        y_ps[:].rearrange("p (n b) -> p n b", b=HB),
                    )
                else:
                    nc.vector.tensor_copy(
                        y_sb[:].rearrange("p (n b) -> p n b", b=BS)[
                            :, 2 * pair:2 * pair + 2, h * HB:(h + 1) * HB],
                        y_ps[:].rearrange("p (n b) -> p n b", b=HB),
                    )

        store_dma = nc.sync.dma_start if STORE_HWDGE else nc.gpsimd.dma_start

        def stores(h, groups):
            n0 = 0
            for g in groups:
                store_dma(
                    y3[:, n0:n0 + g, h * HB:(h + 1) * HB],
                    y_sb[:].rearrange("p (n b) -> p n b", b=BS)[
                        :, n0:n0 + g, h * HB:(h + 1) * HB],
                )
                n0 += g

        # ---- Half-0 phase 2, with half-1's T matmuls as the interleave
        # filler, front-loaded so T1 completes as early as its loads allow;
        # later pairs fall back to clock-keeper dummies. ----
        def inter0(pair):
            lo = min(INTER1_PER_PAIR * pair, NCHUNK)
            hi = min(INTER1_PER_PAIR * (pair + 1), NCHUNK)
            for n in range(lo, hi):
                ph1_mm(1, n)
        ph2(0, inter0)
        # Remaining half-1 T matmuls: a solid block while half-0's stores
        # stream (PE would otherwise idle; all half-1 loads have landed).
        for n in range(INTER1_PER_PAIR * NPAIR, NCHUNK):
            ph1_mm(1, n)
        stores(0, YGROUPS)

        # ---- Half-1 boundary: T1 copy, then phase 2. (No fillers here:
        # the PE still has half-0 pairs queued across this whole stretch,
        # so it never idles into a clock reset, and extra matmuls would
        # only delay the real ones.) ----
        nc.vector.tensor_copy(t_sb[1][:, 0:HB // 2], t_ps[1][:, 0:HB // 2])
        nc.scalar.copy(t_sb[1][:, HB // 2:], t_ps[1][:, HB // 2:])

        def inter1(pair):
            pass
        ph2(1, inter1)
        stores(1, YGROUPS1)


def build_bass():
    import concourse.mybir as mybir
    import concourse.tile as tile
    from concourse import bacc

    # Bacc (not raw Bass): its compile() runs generate_event_semaphores(),
    # which splits multi-sem waits into the 1-wait-per-instruction form the
    # TRN2 ISA requires.
    nc = bacc.Bacc("TRN2", target_bir_lowering=False, debug=False)
    x = nc.dram_tensor("x", [L, BS], mybir.dt.float16, kind="ExternalInput").ap()
    vt = nc.dram_tensor("vt", [P, NCHUNK * RANK], mybir.dt.float16, kind="ExternalInput").ap()
    u = nc.dram_tensor("u", [RANK, L], mybir.dt.float16, kind="ExternalInput").ap()
    y = nc.dram_tensor("y", [L, BS], mybir.dt.float16, kind="ExternalOutput").ap()

    with tile.TileContext(nc) as tc:
        _body(tc, nc, x, vt, u, y, mybir)
    nc.compile()
    return nc


def _get_nc():
    global _NC
    if _NC is None:
        _NC = build_bass()
    return _NC


def make_in_maps(inputs, U, V):
    x = np.asarray(inputs, dtype=np.float32).astype(np.float16)
    U16 = np.ascontiguousarray(np.asarray(U, dtype=np.float32).astype(np.float16))
    V16 = np.asarray(V, dtype=np.float32).astype(np.float16)
    # vt[p, n*RANK + r] = V[r, n*128 + p]
    vt = np.ascontiguousarray(
        V16.reshape(RANK, NCHUNK, P).transpose(2, 1, 0).reshape(P, NCHUNK * RANK)
    )
    in_maps = []
    for c in range(NCORES):
        xs = np.ascontiguousarray(x[:, c * BS:(c + 1) * BS])
        in_maps.append({"x": xs, "vt": vt, "u": U16})
    return in_maps


def kernel(inputs, U, V):
    from concourse import bass_utils

    nc = _get_nc()
    in_maps = make_in_maps(inputs, U, V)
    res = bass_utils.run_bass_kernel_spmd(nc, in_maps, core_ids=list(range(NCORES)))
    return np.concatenate(
        [res.results[c]["y"] for c in range(NCORES)], axis=1
    ).astype(np.float32)
